# revision 1
# baseline (speedup 1.0000x reference)
"""GCN layer kernel for Trainium2 (Bass/Tile), data-parallel over batch.

Reference computation (per batch element):
    deg = A.sum(-1); d = deg ** -0.5
    t   = X @ W.T + b
    out = relu(diag(d) @ A @ diag(d) @ t)

Per-core mapping (8 cores, one batch element each):
  - A streams in as 16 row-tiles [128, 2048] (HWDGE f32 loads), cast
    f32->bf16 on GpSimd (1-input ops run at line rate there), then transposed
    SBUF->SBUF by the DMA xbar (2-byte-only path) into an 8 MB bf16 at_big.
    The tensor engine contracts over partitions, so A's contraction index
    (its column) must live on partitions; the xbar does that off the PE.
    Xbar layout: out[p, 16r + b] = in[r, 128b + p], so the matmul stationary
    for chunk (k-tile b, mu) is a stride-16 AP - no repacking needed.
  - Row degrees reduce on DVE (2x bf16 rate) from the natural bf16 tiles;
    d = sqrt(1/deg) via DVE reciprocal + ACT sqrt.
  - t = X @ W.T in bf16 (X tiles also xbar-transposed; W.T passed
    pre-transposed from host as a layout choice); bias added in f32 from a
    broadcast tile; y = d * t rounded to bf16 by the ACT scale pass.
  - Main matmul accumulates out[mu] = sum_k AT(k,mu).T @ y[k] in PSUM f32:
    8 accumulator banks run during the stream (triangular schedule: product
    (k, mu) is runnable once row-tiles k and mu have both arrived), the
    remaining 8 row-tiles run as a tail batch afterwards.
  - Drain: relu(d * psum) on ACT, then DMA out (f32).
"""

from contextlib import ExitStack

import numpy as np

import concourse.bacc as bacc
import concourse.mybir as mybir
import concourse.tile as tile
from concourse.bass_utils import run_bass_kernel_spmd
from concourse.masks import make_identity

B = 8
N = 2048
F = 256
P = 128
NT = N // P  # 16 row tiles
FT = F // P  # 2 feature tiles
F32 = mybir.dt.float32
BF16 = mybir.dt.bfloat16
COPY = mybir.ActivationFunctionType.Copy
RELU = mybir.ActivationFunctionType.Relu
ACC_SLOTS = 6  # PSUM accumulator banks (2 reserved for transpose staging)


def _emit(ctx: ExitStack, tc: tile.TileContext, A, X, WT, BIAS, OUT):
    nc = tc.nc

    const = ctx.enter_context(tc.tile_pool(name="const", bufs=1))
    stage = ctx.enter_context(tc.tile_pool(name="stage", bufs=4))
    at_pool = ctx.enter_context(tc.tile_pool(name="at", bufs=1))
    outstage = ctx.enter_context(tc.tile_pool(name="outstage", bufs=4))
    psum_acc = ctx.enter_context(
        tc.tile_pool(name="psum_acc", bufs=ACC_SLOTS, space="PSUM")
    )
    psum_tr = ctx.enter_context(tc.tile_pool(name="psum_tr", bufs=2, space="PSUM"))

    ident = const.tile([P, P], BF16, tag="ident")
    make_identity(nc, ident[:, :])
    ident_f32 = const.tile([P, P], F32, tag="identf")
    make_identity(nc, ident_f32[:, :])

    # W.T resident in SBUF as bf16 (f32 HWDGE load + Pool cast)
    wt_stage = const.tile([P, FT * F], F32, tag="wts")
    for phi in range(FT):
        nc.sync.dma_start(
            out=wt_stage[:, phi * F : (phi + 1) * F], in_=WT[phi * P : (phi + 1) * P, :]
        )
    wt_sb = const.tile([P, FT * F], mybir.dt.float32r, tag="wt")
    nc.scalar.copy(wt_sb[:, :], wt_stage[:, :])

    # bias broadcast tile [128, 256] f32 built via ones-column outer product
    b_row = const.tile([1, F], F32, tag="brow")
    nc.sync.dma_start(out=b_row[:, :], in_=BIAS[:, :])
    ones_row = const.tile([1, P], F32, tag="ones")
    nc.vector.memset(ones_row[:, :], 1.0)
    b_psum = psum_acc.tile([P, F], F32, tag="acc", name="b_psum")
    nc.tensor.matmul(b_psum[:, :], ones_row[:, :], b_row[:, :], start=True, stop=True)
    b_bcast = const.tile([P, F], F32, tag="bbc")
    nc.scalar.copy(b_bcast[:, :], b_psum[:, :])

    # degree -> d = sqrt(1/deg) storage, one column per row-tile
    deg = const.tile([P, NT], F32, tag="deg")
    rec = const.tile([P, NT], F32, tag="rec")
    dinv = const.tile([P, NT], F32, tag="dinv")

    # t = X W^T + b in f32; y = bf16 rounded d*t
    t_big = const.tile([P, NT * F], F32, tag="t")
    y_big = const.tile([P, NT * F], BF16, tag="y")

    # transposed adjacency store (xbar 3D-out layout), tile mu at [:, 2048*mu:]:
    # at_big[p, 2048*mu + 128*k + r] = A[128*mu + r, 128*k + p]
    at_big = at_pool.tile([P, NT * N], BF16, tag="at")
    # view [p, mu, k, r]: stationary chunk (k, mu) = at_view[:, mu, k, :] (contiguous)
    at_view = at_big[:, :].rearrange("p (m e r) -> p m e r", m=NT, e=NT)

    # ---- t = X @ W.T + b: one bulk X load, PE f32r transposes in the idle
    # head, mm1 in f32r (2-pass fp32; small). No casts, no xbar. ----
    F32R = mybir.dt.float32r
    xs_f32 = const.tile([P, NT * F], F32, tag="xsf")
    # one DMA: xs_f32[p, 256*mu + f] = X[128*mu + p, f]
    nc.sync.dma_start(
        out=xs_f32[:, :].rearrange("p (m f) -> p m f", m=NT),
        in_=X.rearrange("(m p) f -> p m f", p=P),
    )
    xt_all = const.tile([P, NT * F], F32, tag="xta")
    for mu in range(NT):
        tp = psum_tr.tile([P, 8 * P], BF16, tag="tr", name="xtp")
        tp_f32 = tp[:, : 2 * F].bitcast(F32)  # [128, 256] f32 view of the bank
        for phi in range(FT):
            nc.tensor.transpose(
                tp_f32[:, phi * P : (phi + 1) * P],
                xs_f32[:, (mu * FT + phi) * P : (mu * FT + phi + 1) * P],
                ident_f32[:, :],
            )
        # drain; ACT output rounds to f32r-compatible (f32r mm1 operand)
        nc.scalar.copy(
            xt_all[:, 2 * mu * P : 2 * (mu + 1) * P].bitcast(F32R), tp_f32[:, :]
        )
        t_psum = psum_acc.tile([P, F], F32, tag="acc", name="t_psum")
        for phi in range(FT):
            nc.tensor.matmul(
                t_psum[:, :],
                xt_all[:, (2 * mu + phi) * P : (2 * mu + phi + 1) * P].bitcast(F32R),
                wt_sb[:, phi * F : (phi + 1) * F],
                start=(phi == 0),
                stop=(phi == FT - 1),
            )
        # t + b -> t_big f32 (DVE, PSUM read)
        nc.vector.tensor_add(t_big[:, mu * F : (mu + 1) * F], t_psum[:, :], b_bcast[:, :])

    # ---- stream A row-tiles: degree, d, y, PE transpose, main matmul ----
    PREFETCH = 3
    a_f32_tiles = {}

    def emit_load(j):
        a_f32_tiles[j] = stage.tile([P, N], F32, tag="af", name=f"a_f32_{j}")
        nc.sync.dma_start(out=a_f32_tiles[j][:, :], in_=A[j * P : (j + 1) * P, :])

    for j in range(PREFETCH):
        emit_load(j)

    acc_tiles = {}

    def emit_product(k, mu):
        nc.tensor.matmul(
            acc_tiles[mu][:, :],
            at_view[:, mu, k, :],
            y_big[:, k * F : (k + 1) * F],
            start=(k == 0),
            stop=(k == NT - 1),
        )

    def emit_drain(mu):
        os = outstage.tile([P, F], F32, tag="os")
        nc.scalar.activation(
            os[:, :], acc_tiles[mu][:, :], RELU, scale=dinv[:, mu : mu + 1]
        )
        nc.gpsimd.dma_start(out=OUT[mu * P : (mu + 1) * P, :], in_=os[:, :])

    for i in range(NT):
        if i + PREFETCH < NT:
            emit_load(i + PREFETCH)
        a_f32 = a_f32_tiles.pop(i)
        # one DVE pass: bf16 cast (matmul operand) + row-sum degree accumulator
        a_bf = stage.tile([P, N], BF16, tag="a")
        nc.vector.tensor_scalar(
            out=a_bf[:, :],
            in0=a_f32[:, :],
            scalar1=0.0,
            scalar2=None,
            op0=mybir.AluOpType.add,
            op1=mybir.AluOpType.add,
            accum_out=deg[:, i : i + 1],
        )
        nc.vector.reciprocal(rec[:, i : i + 1], deg[:, i : i + 1])
        nc.scalar.sqrt(dinv[:, i : i + 1], rec[:, i : i + 1])
        # y[i] = d[i] * t[i], rounded to bf16
        nc.scalar.activation(
            y_big[:, i * F : (i + 1) * F],
            t_big[:, i * F : (i + 1) * F],
            COPY,
            scale=dinv[:, i : i + 1],
        )
        # PE transpose-mode (bf16): 8 chunks per PSUM bank, ACT drains to at_big
        for g in range(2):
            tp = psum_tr.tile([P, 8 * P], BF16, tag="tr")
            for j in range(8):
                k = 8 * g + j
                nc.tensor.transpose(
                    tp[:, j * P : (j + 1) * P],
                    a_bf[:, k * P : (k + 1) * P],
                    ident[:, :],
                )
            nc.scalar.copy(
                at_big[:, N * i + 8 * P * g : N * i + 8 * P * (g + 1)], tp[:, :]
            )
        # main-matmul products that just became runnable (early accumulators):
        # every (k, mu) pair with max(k, mu) == i and mu < ACC_SLOTS
        if i < ACC_SLOTS:
            acc_tiles[i] = psum_acc.tile([P, F], F32, tag="acc", name=f"acc_{i}")
            for k in range(i + 1):
                emit_product(k, i)
        for mu in range(min(i, ACC_SLOTS)):
            emit_product(i, mu)

    # ---- drains + tail batches ----
    for mu in range(ACC_SLOTS):
        emit_drain(mu)
    for mu in range(ACC_SLOTS, NT):
        acc_tiles[mu] = psum_acc.tile([P, F], F32, tag="acc", name=f"acc_{mu}")
        for k in range(NT):
            emit_product(k, mu)
        emit_drain(mu)


_cached_nc = None


def _build():
    nc = bacc.Bacc("TRN2", target_bir_lowering=False, debug=False)
    A = nc.dram_tensor("adj", [N, N], F32, kind="ExternalInput").ap()
    X = nc.dram_tensor("x", [N, F], F32, kind="ExternalInput").ap()
    WT = nc.dram_tensor("wt", [F, F], F32, kind="ExternalInput").ap()
    BIAS = nc.dram_tensor("bias", [1, F], F32, kind="ExternalInput").ap()
    OUT = nc.dram_tensor("out", [N, F], F32, kind="ExternalOutput").ap()
    with tile.TileContext(nc) as tc:
        with ExitStack() as ctx:
            _emit(ctx, tc, A, X, WT, BIAS, OUT)
    nc.compile()
    return nc


def get_nc():
    global _cached_nc
    if _cached_nc is None:
        _cached_nc = _build()
    return _cached_nc


def make_in_maps(node_features, adj_matrix, W, b):
    node_features = np.asarray(node_features, dtype=np.float32)
    adj_matrix = np.asarray(adj_matrix, dtype=np.float32)
    wt = np.ascontiguousarray(np.asarray(W, dtype=np.float32).T)
    bias = np.ascontiguousarray(np.asarray(b, dtype=np.float32).reshape(1, F))
    return [
        {
            "adj": np.ascontiguousarray(adj_matrix[c]),
            "x": np.ascontiguousarray(node_features[c]),
            "wt": wt,
            "bias": bias,
        }
        for c in range(B)
    ]


def kernel(node_features, adj_matrix, W, b):
    nc = get_nc()
    in_maps = make_in_maps(node_features, adj_matrix, W, b)
    res = run_bass_kernel_spmd(nc, in_maps, core_ids=list(range(B)))
    return np.stack([r["out"] for r in res.results], axis=0)



# revision 2
# speedup vs baseline: 1.3529x; 1.3529x over previous
"""GCN layer kernel for Trainium2 (Bass/Tile), data-parallel over batch.

Per core (one batch element):
    out = relu(D^-1/2 A D^-1/2 (X W^T + b))

Host-side prep per core (numpy: dtype/layout marshaling + the O(N^2) row-sum):
  - A^T cast to bf16. The tensor engine contracts over partitions, so A's
    contraction index (its column) must live on partitions; shipping A^T makes
    every matmul stationary chunk a contiguous 128-col slice and halves HBM
    traffic vs f32 (the 16 MB f32 A load was the old bottleneck).
  - X^T, W^T, b cast to bf16 (mm1 stationary wants features on partitions).
  - d = deg^-1/2 shipped as a [128, 16] f32 column table (deg = A row sums).
    deg needs full A rows, which live across all 16 A^T tiles on device; doing
    it on host removes a global barrier that would serialize loads vs matmul.

Device schedule (per core):
  - 20 HWDGE loads on the SP ring: X^T/W^T/d/b first, then 16 x 1 MB A^T tiles.
  - mm1 per k-tile: psum = bias (ones-outer-product matmul) + X^T_k @ W^T
    chunks; ACT drains y_k = d_k * psum to bf16. Fills the PE warmup window
    while A^T tiles stream in.
  - main matmul streams per arriving A^T tile k: 16 products
    acc[mu] += AT(k, mu).T @ y_k accumulated over k in PSUM. All 16
    accumulators stay live, packed 2 per bank: one start=True clears the
    bank's has_written bits, the second half-group's first write lands on
    cleared bits (overwrite+set), later writes accumulate per element.
  - drain: relu(d_mu * acc[mu]) on ACT, output DMA on the ACT HWDGE ring.
"""

from contextlib import ExitStack

import ml_dtypes
import numpy as np

import concourse.bacc as bacc
import concourse.mybir as mybir
import concourse.tile as tile
from concourse.bass_utils import run_bass_kernel_spmd

B = 8
N = 2048
F = 256
P = 128
NT = N // P  # 16 row tiles
FT = F // P  # 2 feature tiles
F32 = mybir.dt.float32
BF16 = mybir.dt.bfloat16
COPY = mybir.ActivationFunctionType.Copy
RELU = mybir.ActivationFunctionType.Relu
BF = ml_dtypes.bfloat16


def _emit(ctx: ExitStack, tc: tile.TileContext, AT, XT, WT, BIAS, DCOL, OUT):
    nc = tc.nc

    const = ctx.enter_context(tc.tile_pool(name="const", bufs=1))
    atp = ctx.enter_context(tc.tile_pool(name="atp", bufs=1))
    outstage = ctx.enter_context(tc.tile_pool(name="outstage", bufs=4))
    psum = ctx.enter_context(tc.tile_pool(name="psum", bufs=8, space="PSUM"))

    xt_sb = const.tile([P, FT * N], BF16, tag="xt")
    wt_sb = const.tile([P, FT * F], BF16, tag="wt")
    dcol = const.tile([P, NT], F32, tag="dcol")
    bias_sb = const.tile([1, F], BF16, tag="bias")
    ones1 = const.tile([1, P], BF16, tag="ones")
    y_big = const.tile([P, NT * F], BF16, tag="y")
    at_big = atp.tile([P, NT * N], BF16, tag="at")

    # input DMAs (HWDGE, SP ring): mm1 operands first, then A^T row tiles
    for phi in range(FT):
        nc.sync.dma_start(
            out=xt_sb[:, phi * N : (phi + 1) * N], in_=XT[phi * P : (phi + 1) * P, :]
        )
        nc.sync.dma_start(
            out=wt_sb[:, phi * F : (phi + 1) * F], in_=WT[phi * P : (phi + 1) * P, :]
        )
    nc.sync.dma_start(out=dcol[:, :], in_=DCOL[:, :])
    nc.sync.dma_start(out=bias_sb[:, :], in_=BIAS[:, :])
    for k in range(NT):
        nc.sync.dma_start(
            out=at_big[:, k * N : (k + 1) * N], in_=AT[k * P : (k + 1) * P, :]
        )

    nc.vector.memset(ones1[:, :], 1.0)

    # ---- mm1: t'_k = X_k @ W.T + b (psum), y_k = d_k * t'_k (bf16) ----
    for k in range(NT):
        tp = psum.tile([P, 2 * F], F32, tag="bank", name=f"mm1_{k}")
        nc.tensor.matmul(tp[:, :F], ones1[:, :], bias_sb[:, :], start=True, stop=False)
        for phi in range(FT):
            nc.tensor.matmul(
                tp[:, :F],
                xt_sb[:, phi * N + k * P : phi * N + (k + 1) * P],
                wt_sb[:, phi * F : (phi + 1) * F],
                start=False,
                stop=(phi == FT - 1),
            )
        nc.scalar.activation(
            y_big[:, k * F : (k + 1) * F], tp[:, :F], COPY, scale=dcol[:, k : k + 1]
        )

    # ---- main matmul: acc[mu] = sum_k AT(k, mu).T @ y_k, 2 accs per bank ----
    accs = [
        psum.tile([P, 2 * F], F32, tag="bank", name=f"acc_{bk}") for bk in range(NT // 2)
    ]
    for k in range(NT):
        for mu in range(NT):
            bk, h = divmod(mu, 2)
            nc.tensor.matmul(
                accs[bk][:, h * F : (h + 1) * F],
                at_big[:, k * N + mu * P : k * N + (mu + 1) * P],
                y_big[:, k * F : (k + 1) * F],
                # one bank clear per bank: h==1's first write lands on cleared
                # has_written bits (overwrite+set), so no second start
                start=(k == 0 and h == 0),
                stop=(k == NT - 1),
                skip_group_check=True,
            )

    # ---- drain: relu(d_mu * acc[mu]) -> f32 out (ACT HWDGE ring) ----
    for mu in range(NT):
        bk, h = divmod(mu, 2)
        os = outstage.tile([P, F], F32, tag="os", name=f"os_{mu}")
        nc.scalar.activation(
            os[:, :], accs[bk][:, h * F : (h + 1) * F], RELU, scale=dcol[:, mu : mu + 1]
        )
        nc.scalar.dma_start(out=OUT[mu * P : (mu + 1) * P, :], in_=os[:, :])


_cached_nc = None


def _build():
    nc = bacc.Bacc("TRN2", target_bir_lowering=False, debug=False)
    AT = nc.dram_tensor("at", [N, N], BF16, kind="ExternalInput").ap()
    XT = nc.dram_tensor("xt", [F, N], BF16, kind="ExternalInput").ap()
    WT = nc.dram_tensor("wt", [F, F], BF16, kind="ExternalInput").ap()
    BIAS = nc.dram_tensor("bias", [1, F], BF16, kind="ExternalInput").ap()
    DCOL = nc.dram_tensor("dcol", [P, NT], F32, kind="ExternalInput").ap()
    OUT = nc.dram_tensor("out", [N, F], F32, kind="ExternalOutput").ap()
    with tile.TileContext(nc) as tc:
        with ExitStack() as ctx:
            _emit(ctx, tc, AT, XT, WT, BIAS, DCOL, OUT)
    nc.compile()
    return nc


def get_nc():
    global _cached_nc
    if _cached_nc is None:
        _cached_nc = _build()
    return _cached_nc


def make_in_maps(node_features, adj_matrix, W, b):
    node_features = np.asarray(node_features, dtype=np.float32)
    adj_matrix = np.asarray(adj_matrix, dtype=np.float32)
    wt = np.ascontiguousarray(np.asarray(W, dtype=np.float32).T.astype(BF))
    bias = np.ascontiguousarray(
        np.asarray(b, dtype=np.float32).astype(BF).reshape(1, F)
    )
    maps = []
    for c in range(B):
        adj = adj_matrix[c]
        deg = adj.sum(axis=1, dtype=np.float32)
        with np.errstate(divide="ignore"):
            d = deg**-0.5
        d = np.where(np.isfinite(d), d, 0.0).astype(np.float32)
        maps.append(
            {
                "at": np.ascontiguousarray(adj.astype(BF).T),
                "xt": np.ascontiguousarray(node_features[c].T.astype(BF)),
                "wt": wt,
                "bias": bias,
                "dcol": np.ascontiguousarray(d.reshape(NT, P).T),
            }
        )
    return maps


def kernel(node_features, adj_matrix, W, b):
    nc = get_nc()
    in_maps = make_in_maps(node_features, adj_matrix, W, b)
    res = run_bass_kernel_spmd(nc, in_maps, core_ids=list(range(B)))
    return np.stack([r["out"] for r in res.results], axis=0)
